# revision 18
# baseline (speedup 1.0000x reference)
"""TRN2 Bass kernel for nn_Att_block_45183055954204.

Gated-attention block: dual layernorm + gated q/k/v + TxT softmax attention
+ residual. Data-parallel over batch B=16 across 8 NeuronCores (2 batches
per core), weights replicated, no collectives.

Math restructure (exact, given softmax rows sum to 1):
  xn       = (x - mean)/sqrt(var + eps)                  (row stats)
  kT[c,t]  = xn[t,c]*gk[c] + bk[c],   gk = ln2_g*sig(kp), bk = ln2_b*sig(kp)
  qT       = A2^T @ kT + bq3   (q-projection folded onto kT; A2, bq3 host-
                                precomputed from Wq, ln1_g/b, bq, sig(qp), gk, bk)
  ST       = kT^T-blocks @ qT  ->  S^T[ts, tq]  (scores, transposed layout)
  ET       = exp(ST / sqrt(C))                  (no max-sub: |S|<~6 always)
  U'       = ET^T-blocks @ xn_rowmajor         (gv folded into output step)
  sums     = bf16(ET)^T-blocks @ ones           (softmax denominators)
  y        = q_rowmajor + (U'/sums)*gv + bv     (bv = ln2_b*vg; sum(P)=1 term)
All layouts chosen so every reduction is a matmul contraction or a
free-dim op; the only transposes are PE-transposes of xn and q.
"""
import math
import numpy as np
import ml_dtypes

C = 512
T = 2048
B = 16
N_CORES = 8
B_LOC = B // N_CORES
EPS = 1e-5

_RUNNER_CACHE = {}


def _sigmoid(z):
    return 1.0 / (1.0 + np.exp(-z))


def build_program(b_loc=B_LOC, t=T, use_f32r=True, with_reps=True,
                  kt_eng='dve', sq_eng='act', xn_eng='dve', o1_eng='dve',
                  qt_eng='act', parts='full'):
    import concourse.bass as bass
    import concourse.tile as tile
    import concourse.bacc as bacc
    from concourse import mybir
    from contextlib import ExitStack

    F32 = mybir.dt.float32
    F32R = mybir.dt.float32r
    I32 = mybir.dt.int32
    AF = mybir.ActivationFunctionType
    ALU = mybir.AluOpType

    NT = t // 128          # 128-row chunks per batch
    NB = t // 512          # 512-row blocks per batch
    rC = 1.0 / C
    MMDT = F32R if use_f32r else F32

    nc = bacc.Bacc("TRN2", debug=False, target_bir_lowering=False)

    xs = nc.dram_tensor("xs", [b_loc, t, C], F32, kind="ExternalInput")
    A2d = nc.dram_tensor("A2", [C, C], MMDT, kind="ExternalInput")
    bq3d = nc.dram_tensor("bq3", [128, 4], F32, kind="ExternalInput")
    gkd = nc.dram_tensor("gk", [128, 4], F32, kind="ExternalInput")
    bkd = nc.dram_tensor("bk", [128, 4], F32, kind="ExternalInput")
    gvd = nc.dram_tensor("gv", [128, C], F32, kind="ExternalInput")
    bvd = nc.dram_tensor("bv", [128, C], F32, kind="ExternalInput")
    idd = nc.dram_tensor("ident", [128, 128], F32, kind="ExternalInput")
    idrd = nc.dram_tensor("identr", [128, 128], MMDT, kind="ExternalInput")
    oned = nc.dram_tensor("ones", [128, 1], mybir.dt.bfloat16, kind="ExternalInput")
    if with_reps:
        repd = nc.dram_tensor("reps", [1, 1], I32, kind="ExternalInput")
    ys = nc.dram_tensor("ys", [b_loc, t, C], F32, kind="ExternalOutput")

    with tile.TileContext(nc) as tc, ExitStack() as ctx:
        # ---- persistent pools -------------------------------------------
        wpool = ctx.enter_context(tc.tile_pool(name="weights", bufs=1))
        kpool = ctx.enter_context(tc.tile_pool(name="kT", bufs=1))
        qpool = ctx.enter_context(tc.tile_pool(name="qT", bufs=1))
        vpool = ctx.enter_context(tc.tile_pool(name="vt", bufs=1))
        # ---- working pools ----------------------------------------------
        xpool = ctx.enter_context(tc.tile_pool(name="x", bufs=3))
        spool = ctx.enter_context(tc.tile_pool(name="stats", bufs=3))
        npool = ctx.enter_context(tc.tile_pool(name="xn", bufs=3))
        epool = ctx.enter_context(tc.tile_pool(name="et", bufs=4))
        opool = ctx.enter_context(tc.tile_pool(name="outs", bufs=3))
        # PSUM: 8 banks total. tag "mm" (tr/qp/st/qrm) 3 + "up" 4 + "sums" 1.
        psum = ctx.enter_context(tc.tile_pool(name="psum", bufs=3, space="PSUM"))
        ps_up = ctx.enter_context(tc.tile_pool(name="ps_up", bufs=4, space="PSUM"))
        ps_sm = ctx.enter_context(tc.tile_pool(name="ps_sm", bufs=1, space="PSUM"))

        # ---- load weights/constants -------------------------------------
        A2 = []
        for ci in range(4):
            w = wpool.tile([128, C], MMDT, tag=f"A2_{ci}")
            nc.sync.dma_start(w[:], A2d[128 * ci:128 * (ci + 1), :])
            A2.append(w)
        bq3 = wpool.tile([128, 4], F32, tag="bq3")
        nc.sync.dma_start(bq3[:], bq3d[:, :])
        gk = wpool.tile([128, 4], F32, tag="gk")
        nc.sync.dma_start(gk[:], gkd[:, :])
        bk = wpool.tile([128, 4], F32, tag="bk")
        nc.sync.dma_start(bk[:], bkd[:, :])
        gv = wpool.tile([128, C], F32, tag="gv")
        nc.sync.dma_start(gv[:], gvd[:, :])
        bv = wpool.tile([128, C], F32, tag="bv")
        nc.sync.dma_start(bv[:], bvd[:, :])
        ident = wpool.tile([128, 128], F32, tag="ident")
        nc.sync.dma_start(ident[:], idd[:, :])
        identr = wpool.tile([128, 128], MMDT, tag="identr")
        nc.sync.dma_start(identr[:], idrd[:, :])
        ones = wpool.tile([128, 1], mybir.dt.bfloat16, tag="ones")
        nc.sync.dma_start(ones[:], oned[:, :])
        eps_t = wpool.tile([128, 1], F32, tag="eps")
        nc.vector.memset(eps_t[:], EPS)

        kT = [kpool.tile([128, t], MMDT, tag=f"kT{ci}", name=f"kT{ci}")
              for ci in range(4)]
        qT = [qpool.tile([128, t], MMDT, tag=f"qT{ci}", name=f"qT{ci}")
              for ci in range(4)]
        xn_rm = vpool.tile([128, 4 * t], MMDT, tag="xn_rm")

        def body():
            if parts == 'dmaonly':
                for b in range(b_loc):
                    for i in range(NT):
                        x_t = xpool.tile([128, C], F32, tag="x")
                        nc.sync.dma_start(x_t[:],
                                          xs[b, 128 * i:128 * (i + 1), :])
                        y2 = opool.tile([128, C], F32, tag="y2")
                        nc.vector.tensor_tensor(y2[:], x_t[:], gv[:],
                                                ALU.mult)
                        nc.sync.dma_start(ys[b, 128 * i:128 * (i + 1), :],
                                          y2[:])
                return
            for b in range(b_loc):
                # ======== Phase A: layernorm, vtilde, kT =================
                for i in range(NT):
                    x_t = xpool.tile([128, C], F32, tag="x")
                    nc.sync.dma_start(x_t[:], xs[b, 128 * i:128 * (i + 1), :])
                    s1 = spool.tile([128, 1], F32, tag="s1")
                    nc.vector.reduce_sum(s1[:], x_t[:], axis=mybir.AxisListType.X)
                    sqr = npool.tile([128, C], F32, tag="sqr")
                    s2 = spool.tile([128, 1], F32, tag="s2")
                    if sq_eng == 'act':
                        nc.scalar.activation(sqr[:], x_t[:], AF.Square,
                                             accum_out=s2[:])
                    else:
                        nc.vector.tensor_tensor_reduce(
                            sqr[:], x_t[:], x_t[:], 1.0, 0.0,
                            ALU.mult, ALU.add, s2[:])
                    t2 = spool.tile([128, 1], F32, tag="t2")
                    nc.vector.tensor_tensor(t2[:], s1[:], s1[:], ALU.mult)
                    vn = spool.tile([128, 1], F32, tag="vn")
                    nc.vector.scalar_tensor_tensor(
                        vn[:], t2[:], -rC, s2[:], ALU.mult, ALU.add)
                    sd = spool.tile([128, 1], F32, tag="sd")
                    nc.scalar.activation(sd[:], vn[:], AF.Sqrt,
                                         bias=eps_t[:], scale=rC)
                    rstd = spool.tile([128, 1], F32, tag="rstd")
                    nc.vector.reciprocal(rstd[:], sd[:])
                    nb = spool.tile([128, 1], F32, tag="nb")
                    nc.vector.scalar_tensor_tensor(
                        nb[:], s1[:], -rC, rstd[:], ALU.mult, ALU.mult)
                    xn = xn_rm[:, C * i:C * (i + 1)]
                    if xn_eng == 'act':
                        nc.scalar.activation(xn, x_t[:], AF.Identity,
                                             bias=nb[:], scale=rstd[:])
                    else:
                        nc.vector.tensor_scalar(
                            xn, x_t[:], rstd[:], nb[:],
                            ALU.mult, ALU.add)
                    tr = psum.tile([128, C], MMDT, tag="mm")
                    for ci in range(4):
                        nc.tensor.transpose(tr[:, 128 * ci:128 * (ci + 1)],
                                            xn[:, 128 * ci:128 * (ci + 1)],
                                            identr[:])
                    for ci in range(4):
                        use_dve = (kt_eng == 'dve' or
                                   (kt_eng == 'split' and ci < 2))
                        if use_dve:
                            nc.vector.tensor_scalar(
                                kT[ci][:, 128 * i:128 * (i + 1)],
                                tr[:, 128 * ci:128 * (ci + 1)],
                                gk[:, ci:ci + 1], bk[:, ci:ci + 1],
                                ALU.mult, ALU.add)
                        else:
                            nc.scalar.activation(
                                kT[ci][:, 128 * i:128 * (i + 1)],
                                tr[:, 128 * ci:128 * (ci + 1)],
                                AF.Identity,
                                bias=bk[:, ci:ci + 1], scale=gk[:, ci:ci + 1])

                # ======== Phase B: qT = A2^T @ kT + bq3 ==================
                for co in range(4):
                    for tb in range(NB):
                        qp = psum.tile([128, 512], F32, tag="mm")
                        for ci in range(4):
                            nc.tensor.matmul(
                                qp[:],
                                A2[ci][:, 128 * co:128 * (co + 1)],
                                kT[ci][:, 512 * tb:512 * (tb + 1)],
                                start=(ci == 0), stop=(ci == 3))
                        if qt_eng == 'act':
                            nc.scalar.activation(
                                qT[co][:, 512 * tb:512 * (tb + 1)], qp[:],
                                AF.Identity, bias=bq3[:, co:co + 1])
                        else:
                            nc.vector.tensor_scalar(
                                qT[co][:, 512 * tb:512 * (tb + 1)], qp[:],
                                bq3[:, co:co + 1], None, ALU.add)

                if parts == 'noc':
                    for i in range(NT):
                        y2 = opool.tile([128, C], F32, tag="y2")
                        nc.vector.tensor_tensor(
                            y2[:], xn_rm[:, C * i:C * (i + 1)], gv[:],
                            ALU.mult)
                        nc.sync.dma_start(ys[b, 128 * i:128 * (i + 1), :],
                                          y2[:])
                    continue
                # ======== Phase C: attention per 512-row q block =========
                for tb in range(NB):
                    up = [ps_up.tile([128, 512], F32, tag="up",
                                           name=f"up{mi}")
                          for mi in range(4)]
                    sums = ps_sm.tile([128, 4], F32, tag="sums")
                    if parts == 'nosums':
                        nc.vector.memset(sums[:], 1.0)
                    ets = {}

                    def st_exp(j, tb=tb):
                        st = psum.tile([128, 512], F32, tag="mm",
                                       name="st")
                        for ci in range(4):
                            nc.tensor.matmul(
                                st[:],
                                kT[ci][:, 128 * j:128 * (j + 1)],
                                qT[ci][:, 512 * tb:512 * (tb + 1)],
                                start=(ci == 0), stop=(ci == 3))
                        et = epool.tile([128, 512], MMDT, tag="et",
                                        name="et")
                        nc.scalar.activation(et[:], st[:], AF.Exp,
                                             scale=1.0 / math.sqrt(C))
                        etb = epool.tile([128, 512], mybir.dt.bfloat16,
                                         tag="etb", name="etb")
                        nc.vector.tensor_copy(etb[:], et[:])
                        ets[j] = (et, etb)

                    st_exp(0)
                    if NT > 1:
                        st_exp(1)
                    for j in range(NT):
                        if j + 2 < NT:
                            st_exp(j + 2)
                        et, etb = ets.pop(j)
                        for mi in range(4):
                            nc.tensor.matmul(
                                up[mi][:],
                                et[:, 128 * mi:128 * (mi + 1)],
                                xn_rm[:, 512 * j:512 * (j + 1)],
                                start=(j == 0), stop=(j == NT - 1))
                            if parts != 'nosums':
                                nc.tensor.matmul(
                                    sums[:, mi:mi + 1],
                                    etb[:, 128 * mi:128 * (mi + 1)],
                                    ones[:],
                                    start=(j == 0 and mi == 0),
                                    stop=(j == NT - 1 and mi == 3))
                    recip = spool.tile([128, 4], F32, tag="recip")
                    nc.vector.reciprocal(recip[:], sums[:])
                    for mi in range(4):
                        qrm = psum.tile([128, 512], MMDT, tag="mm")
                        for ci in range(4):
                            nc.tensor.transpose(
                                qrm[:, 128 * ci:128 * (ci + 1)],
                                qT[ci][:, 512 * tb + 128 * mi:
                                       512 * tb + 128 * (mi + 1)],
                                identr[:])
                        o1 = opool.tile([128, 512], F32, tag="o1")
                        if o1_eng == 'act':
                            # t = up * recip (per-partition) on ACT, then
                            # o1 = t * gv_row on DVE
                            t_ = opool.tile([128, 512], F32, tag="t_")
                            nc.scalar.activation(
                                t_[:], up[mi][:], AF.Identity,
                                scale=recip[:, mi:mi + 1])
                            nc.vector.tensor_tensor(o1[:], t_[:], gv[:],
                                                    ALU.mult)
                        else:
                            nc.vector.scalar_tensor_tensor(
                                o1[:], up[mi][:],
                                recip[:, mi:mi + 1], gv[:],
                                ALU.mult, ALU.mult)
                        y1 = opool.tile([128, 512], F32, tag="y1")
                        nc.vector.tensor_tensor(y1[:], qrm[:], bv[:], ALU.add)
                        y2 = opool.tile([128, 512], F32, tag="y2")
                        nc.vector.tensor_tensor(y2[:], y1[:], o1[:], ALU.add)
                        row0 = 512 * tb + 128 * mi
                        nc.sync.dma_start(ys[b, row0:row0 + 128, :], y2[:])

        if with_reps:
            rp = wpool.tile([1, 1], I32, tag="reps")
            nc.sync.dma_start(rp[:], repd[:, :])
            n_reps = nc.values_load(rp[0:1, 0:1].to_broadcast((1, 1)))
            with tc.For_i(0, n_reps, 1):
                body()
        else:
            body()

    nc.compile()
    return nc


def prepare_weights(ln1_g, ln1_b, ln2_g, ln2_b, qp, kp, vp, Wq, bq,
                    Wv1, bv1, Wv2, bv2):
    """Host-side folding of all parameter-only math (all fp64 -> fp32)."""
    f8 = np.float64
    ln1_g, ln1_b = f8(ln1_g), f8(ln1_b)
    ln2_g, ln2_b = f8(ln2_g), f8(ln2_b)
    qp, kp, vp = f8(qp), f8(kp), f8(vp)
    Wq, bq = f8(Wq), f8(bq)
    sq = _sigmoid(qp)[0]
    sk = _sigmoid(kp)[0]
    vg = (_sigmoid(vp @ f8(Wv1).T + f8(bv1)) *
          np.tanh(vp @ f8(Wv2).T + f8(bv2)))[0]
    gk = ln2_g * sk
    bk = ln2_b * sk
    gv = ln2_g * vg
    bv = ln2_b * vg
    # q = (xn*g1 + b1) @ Wq.T * sq + bq*sq  => A[c,d] = g1[c]*Wq[d,c]*sq[d]
    A = ln1_g[:, None] * Wq.T * sq[None, :]
    bq2 = (Wq @ ln1_b + bq) * sq
    # substitute xn = (kT - bk)/gk
    A2 = A / gk[:, None]
    bq3 = bq2 - A2.T @ bk

    def pack4(v):
        return np.ascontiguousarray(v.reshape(4, 128).T).astype(np.float32)

    return {
        "A2": A2.astype(np.float32),
        "bq3": pack4(bq3),
        "gk": pack4(gk),
        "bk": pack4(bk),
        "gv": np.broadcast_to(gv.astype(np.float32), (128, C)).copy(),
        "bv": np.broadcast_to(bv.astype(np.float32), (128, C)).copy(),
        "ident": np.eye(128, dtype=np.float32),
        "identr": np.eye(128, dtype=np.float32),
        "ones": np.ones((128, 1), dtype=ml_dtypes.bfloat16),
    }


def get_runner(b_loc=B_LOC, t=T, use_f32r=True, with_reps=True, **bkw):
    """Build + jit once; returns run(in_maps) -> list of per-core out dicts.

    Mirrors bass2jax.run_bass_via_pjrt's shard_map path but keeps the jitted
    callable so repeated executions (timing) don't re-trace/re-compile.
    """
    key = (b_loc, t, use_f32r, with_reps, tuple(sorted(bkw.items())))
    if key in _RUNNER_CACHE:
        return _RUNNER_CACHE[key]

    import jax
    from jax.sharding import Mesh, PartitionSpec
    from jax.experimental.shard_map import shard_map
    from concourse import bass2jax, mybir
    from concourse.bass2jax import _bass_exec_p, partition_id_tensor

    nc = build_program(b_loc=b_loc, t=t, use_f32r=use_f32r,
                       with_reps=with_reps, **bkw)
    bass2jax.install_neuronx_cc_hook()

    partition_name = (nc.partition_id_tensor.name
                      if nc.partition_id_tensor else None)
    in_names, out_names, out_avals, zero_shapes = [], [], [], []
    for alloc in nc.m.functions[0].allocations:
        if not isinstance(alloc, mybir.MemoryLocationSet):
            continue
        name = alloc.memorylocations[0].name
        if alloc.kind == "ExternalInput":
            if name != partition_name:
                in_names.append(name)
        elif alloc.kind == "ExternalOutput":
            shape = tuple(alloc.tensor_shape)
            dtype = mybir.dt.np(alloc.dtype)
            out_names.append(name)
            out_avals.append(jax.core.ShapedArray(shape, dtype))
            zero_shapes.append((shape, dtype))
    n_params = len(in_names)
    n_outs = len(out_names)
    all_in = list(in_names) + list(out_names)
    if partition_name is not None:
        all_in.append(partition_name)

    def _body(*args):
        operands = list(args)
        if partition_name is not None:
            operands.append(partition_id_tensor())
        outs = _bass_exec_p.bind(
            *operands,
            out_avals=tuple(out_avals),
            in_names=tuple(all_in),
            out_names=tuple(out_names),
            lowering_input_output_aliases=(),
            sim_require_finite=True,
            sim_require_nnan=True,
            nc=nc,
        )
        return tuple(outs)

    devices = jax.devices()[:N_CORES]
    mesh = Mesh(np.asarray(devices), ("core",))
    in_specs = (PartitionSpec("core"),) * (n_params + n_outs)
    out_specs = (PartitionSpec("core"),) * n_outs
    donate = tuple(range(n_params, n_params + n_outs))
    sharded = jax.jit(
        shard_map(_body, mesh=mesh, in_specs=in_specs, out_specs=out_specs,
                  check_rep=False),
        donate_argnums=donate, keep_unused=True)

    def run(in_maps):
        per_core = [[np.asarray(m[name]) for name in in_names]
                    for m in in_maps]
        concat_in = [
            np.concatenate([per_core[c][i] for c in range(N_CORES)], axis=0)
            for i in range(n_params)
        ]
        concat_zeros = [
            np.zeros((N_CORES * s[0], *s[1:]), d) for (s, d) in zero_shapes
        ]
        out_arrs = sharded(*concat_in, *concat_zeros)
        out_arrs = [np.asarray(a) for a in out_arrs]
        return [
            {name: out_arrs[i].reshape(N_CORES, *out_avals[i].shape)[c]
             for i, name in enumerate(out_names)}
            for c in range(N_CORES)
        ]

    _RUNNER_CACHE[key] = (run, nc)
    return run, nc


def make_in_maps(x, weights, reps=1, with_reps=True):
    maps = []
    for c in range(N_CORES):
        m = {"xs": np.ascontiguousarray(
            x[c * B_LOC:(c + 1) * B_LOC]).astype(np.float32)}
        m.update(weights)
        if with_reps:
            m["reps"] = np.array([[reps]], dtype=np.int32)
        maps.append(m)
    return maps


def kernel(x, ln1_g, ln1_b, ln2_g, ln2_b, qp, kp, vp, Wq, bq,
           Wv1, bv1, Wv2, bv2):
    x = np.asarray(x, dtype=np.float32)
    weights = prepare_weights(ln1_g, ln1_b, ln2_g, ln2_b, qp, kp, vp,
                              Wq, bq, Wv1, bv1, Wv2, bv2)
    run, _ = get_runner()
    in_maps = make_in_maps(x, weights)
    results = run(in_maps)
    out = np.concatenate([results[c]["ys"] for c in range(N_CORES)], axis=0)
    return out.astype(np.float32)


# revision 21
# speedup vs baseline: 4.6646x; 4.6646x over previous
"""TRN2 Bass kernel for nn_Att_block_45183055954204.

Gated-attention block: dual layernorm + gated q/k/v + TxT softmax attention
+ residual. Data-parallel over batch B=16 across 8 NeuronCores (2 batches
per core), weights replicated, no collectives.

Math restructure (exact, given softmax rows sum to 1):
  xn       = (x - mean)/sqrt(var + eps)                  (row stats)
  kT[c,t]  = xn[t,c]*gk[c] + bk[c],   gk = ln2_g*sig(kp), bk = ln2_b*sig(kp)
  qT       = A2^T @ kT + bq3   (q-projection folded onto kT; A2, bq3 host-
                                precomputed from Wq, ln1_g/b, bq, sig(qp), gk, bk)
  ST       = kT^T-blocks @ qT  ->  S^T[ts, tq]  (scores, transposed layout)
  ET       = exp(ST / sqrt(C))                  (no max-sub: |S|<~6 always)
  U'       = ET^T-blocks @ xn_rowmajor         (gv folded into output step)
  sums     = bf16(ET)^T-blocks @ ones           (softmax denominators)
  y        = q_rowmajor + (U'/sums)*gv + bv     (bv = ln2_b*vg; sum(P)=1 term)
All layouts chosen so every reduction is a matmul contraction or a
free-dim op; the only transposes are PE-transposes of xn and q.
"""
import math
import numpy as np
import ml_dtypes

C = 512
T = 2048
B = 16
N_CORES = 8
B_LOC = B // N_CORES
EPS = 1e-5

_RUNNER_CACHE = {}


def _sigmoid(z):
    return 1.0 / (1.0 + np.exp(-z))


def build_program(b_loc=B_LOC, t=T, use_f32r=True, with_reps=True,
                  kt_eng='dve', sq_eng='act', xn_eng='dve', o1_eng='dve',
                  qt_eng='act', parts='full'):
    import concourse.bass as bass
    import concourse.tile as tile
    import concourse.bacc as bacc
    from concourse import mybir
    from contextlib import ExitStack

    F32 = mybir.dt.float32
    F32R = mybir.dt.float32r
    I32 = mybir.dt.int32
    AF = mybir.ActivationFunctionType
    ALU = mybir.AluOpType

    NT = t // 128          # 128-row chunks per batch
    NB = t // 512          # 512-row blocks per batch
    rC = 1.0 / C
    MMDT = F32R if use_f32r else F32

    nc = bacc.Bacc("TRN2", debug=False, target_bir_lowering=False)

    xs = nc.dram_tensor("xs", [b_loc, t, C], F32, kind="ExternalInput")
    A2d = nc.dram_tensor("A2", [C, C], MMDT, kind="ExternalInput")
    bq3d = nc.dram_tensor("bq3", [128, 4], F32, kind="ExternalInput")
    gkd = nc.dram_tensor("gk", [128, 4], F32, kind="ExternalInput")
    bkd = nc.dram_tensor("bk", [128, 4], F32, kind="ExternalInput")
    gvd = nc.dram_tensor("gv", [128, C], F32, kind="ExternalInput")
    bvd = nc.dram_tensor("bv", [128, C], F32, kind="ExternalInput")
    bvqd = nc.dram_tensor("bvq", [128, C], F32, kind="ExternalInput")
    idd = nc.dram_tensor("ident", [128, 128], F32, kind="ExternalInput")
    idrd = nc.dram_tensor("identr", [128, 128], MMDT, kind="ExternalInput")
    oned = nc.dram_tensor("ones", [128, 1], mybir.dt.bfloat16, kind="ExternalInput")
    if with_reps:
        repd = nc.dram_tensor("reps", [1, 1], I32, kind="ExternalInput")
    ys = nc.dram_tensor("ys", [b_loc, t, C], F32, kind="ExternalOutput")

    with tile.TileContext(nc) as tc, ExitStack() as ctx:
        # ---- persistent pools -------------------------------------------
        wpool = ctx.enter_context(tc.tile_pool(name="weights", bufs=1))
        kpool = ctx.enter_context(tc.tile_pool(name="kT", bufs=1))
        qpool = ctx.enter_context(tc.tile_pool(name="qT", bufs=1))
        vpool = ctx.enter_context(tc.tile_pool(name="vt", bufs=1))
        # ---- working pools ----------------------------------------------
        xpool = ctx.enter_context(tc.tile_pool(name="x", bufs=3))
        spool = ctx.enter_context(tc.tile_pool(name="stats", bufs=3))
        npool = ctx.enter_context(tc.tile_pool(name="xn", bufs=3))
        epool = ctx.enter_context(tc.tile_pool(name="et", bufs=4))
        opool = ctx.enter_context(tc.tile_pool(name="outs", bufs=3))
        # PSUM: 8 banks total. tag "mm" (tr/qp/st/qrm) 3 + "up" 4 + "sums" 1.
        psum = ctx.enter_context(tc.tile_pool(name="psum", bufs=3, space="PSUM"))
        ps_up = ctx.enter_context(tc.tile_pool(name="ps_up", bufs=4, space="PSUM"))
        ps_sm = ctx.enter_context(tc.tile_pool(name="ps_sm", bufs=1, space="PSUM"))
        dpool = ctx.enter_context(tc.tile_pool(name="dscratch", bufs=2,
                                               space="DRAM"))

        # ---- load weights/constants -------------------------------------
        A2 = []
        for ci in range(4):
            w = wpool.tile([128, C], MMDT, tag=f"A2_{ci}")
            nc.sync.dma_start(w[:], A2d[128 * ci:128 * (ci + 1), :])
            A2.append(w)
        bq3 = wpool.tile([128, 4], F32, tag="bq3")
        nc.sync.dma_start(bq3[:], bq3d[:, :])
        gk = wpool.tile([128, 4], F32, tag="gk")
        nc.sync.dma_start(gk[:], gkd[:, :])
        bk = wpool.tile([128, 4], F32, tag="bk")
        nc.sync.dma_start(bk[:], bkd[:, :])
        gv = wpool.tile([128, C], F32, tag="gv")
        nc.sync.dma_start(gv[:], gvd[:, :])
        bv = wpool.tile([128, C], F32, tag="bv")
        nc.sync.dma_start(bv[:], bvd[:, :])
        bvq = wpool.tile([128, C], F32, tag="bvq")
        nc.sync.dma_start(bvq[:], bvqd[:, :])
        ident = wpool.tile([128, 128], F32, tag="ident")
        nc.sync.dma_start(ident[:], idd[:, :])
        identr = wpool.tile([128, 128], MMDT, tag="identr")
        nc.sync.dma_start(identr[:], idrd[:, :])
        ones = wpool.tile([128, 1], mybir.dt.bfloat16, tag="ones")
        nc.sync.dma_start(ones[:], oned[:, :])
        eps_t = wpool.tile([128, 1], F32, tag="eps")
        nc.vector.memset(eps_t[:], EPS)

        kT = [kpool.tile([128, t], MMDT, tag=f"kT{ci}", name=f"kT{ci}")
              for ci in range(4)]
        qT = [qpool.tile([128, t], MMDT, tag=f"qT{ci}", name=f"qT{ci}")
              for ci in range(4)]
        xn_rm = vpool.tile([128, 4 * t], MMDT, tag="xn_rm")

        def body():
            if parts == 'dmaonly':
                for b in range(b_loc):
                    for i in range(NT):
                        x_t = xpool.tile([128, C], F32, tag="x")
                        nc.sync.dma_start(x_t[:],
                                          xs[b, 128 * i:128 * (i + 1), :])
                        y2 = opool.tile([128, C], F32, tag="y2")
                        nc.vector.tensor_tensor(y2[:], x_t[:], gv[:],
                                                ALU.mult)
                        nc.sync.dma_start(ys[b, 128 * i:128 * (i + 1), :],
                                          y2[:])
                return
            for b in range(b_loc):
                # ======== Phase A: layernorm, vtilde, kT =================
                for i in range(NT):
                    x_t = xpool.tile([128, C], F32, tag="x")
                    nc.sync.dma_start(x_t[:], xs[b, 128 * i:128 * (i + 1), :])
                    s1 = spool.tile([128, 1], F32, tag="s1")
                    nc.vector.reduce_sum(s1[:], x_t[:], axis=mybir.AxisListType.X)
                    sqr = npool.tile([128, C], F32, tag="sqr")
                    s2 = spool.tile([128, 1], F32, tag="s2")
                    if sq_eng == 'act':
                        nc.scalar.activation(sqr[:], x_t[:], AF.Square,
                                             accum_out=s2[:])
                    else:
                        nc.vector.tensor_tensor_reduce(
                            sqr[:], x_t[:], x_t[:], 1.0, 0.0,
                            ALU.mult, ALU.add, s2[:])
                    t2 = spool.tile([128, 1], F32, tag="t2")
                    nc.vector.tensor_tensor(t2[:], s1[:], s1[:], ALU.mult)
                    vn = spool.tile([128, 1], F32, tag="vn")
                    nc.vector.scalar_tensor_tensor(
                        vn[:], t2[:], -rC, s2[:], ALU.mult, ALU.add)
                    sd = spool.tile([128, 1], F32, tag="sd")
                    nc.scalar.activation(sd[:], vn[:], AF.Sqrt,
                                         bias=eps_t[:], scale=rC)
                    rstd = spool.tile([128, 1], F32, tag="rstd")
                    nc.vector.reciprocal(rstd[:], sd[:])
                    nb = spool.tile([128, 1], F32, tag="nb")
                    nc.vector.scalar_tensor_tensor(
                        nb[:], s1[:], -rC, rstd[:], ALU.mult, ALU.mult)
                    xn = xn_rm[:, C * i:C * (i + 1)]
                    if xn_eng == 'act':
                        nc.scalar.activation(xn, x_t[:], AF.Identity,
                                             bias=nb[:], scale=rstd[:])
                    else:
                        nc.vector.tensor_scalar(
                            xn, x_t[:], rstd[:], nb[:],
                            ALU.mult, ALU.add)
                    tr = psum.tile([128, C], MMDT, tag="mm")
                    for ci in range(4):
                        nc.tensor.transpose(tr[:, 128 * ci:128 * (ci + 1)],
                                            xn[:, 128 * ci:128 * (ci + 1)],
                                            identr[:])
                    for ci in range(4):
                        use_dve = (kt_eng == 'dve' or
                                   (kt_eng == 'split' and ci < 2))
                        if use_dve:
                            nc.vector.tensor_scalar(
                                kT[ci][:, 128 * i:128 * (i + 1)],
                                tr[:, 128 * ci:128 * (ci + 1)],
                                gk[:, ci:ci + 1], bk[:, ci:ci + 1],
                                ALU.mult, ALU.add)
                        else:
                            nc.scalar.activation(
                                kT[ci][:, 128 * i:128 * (i + 1)],
                                tr[:, 128 * ci:128 * (ci + 1)],
                                AF.Identity,
                                bias=bk[:, ci:ci + 1], scale=gk[:, ci:ci + 1])

                # ======== Phase B: qT = A2^T @ kT + bq3 ==================
                for co in range(4):
                    for tb in range(NB):
                        qp = psum.tile([128, 512], F32, tag="mm")
                        for ci in range(4):
                            nc.tensor.matmul(
                                qp[:],
                                A2[ci][:, 128 * co:128 * (co + 1)],
                                kT[ci][:, 512 * tb:512 * (tb + 1)],
                                start=(ci == 0), stop=(ci == 3))
                        if qt_eng == 'act':
                            nc.scalar.activation(
                                qT[co][:, 512 * tb:512 * (tb + 1)], qp[:],
                                AF.Identity, bias=bq3[:, co:co + 1])
                        else:
                            nc.vector.tensor_scalar(
                                qT[co][:, 512 * tb:512 * (tb + 1)], qp[:],
                                bq3[:, co:co + 1], None, ALU.add)

                if parts == 'noc':
                    for i in range(NT):
                        y2 = opool.tile([128, C], F32, tag="y2")
                        nc.vector.tensor_tensor(
                            y2[:], xn_rm[:, C * i:C * (i + 1)], gv[:],
                            ALU.mult)
                        nc.sync.dma_start(ys[b, 128 * i:128 * (i + 1), :],
                                          y2[:])
                    continue
                # ======== Phase C: attention per 512-row q block =========
                for tb in range(NB):
                    up = [ps_up.tile([128, 512], F32, tag="up",
                                           name=f"up{mi}")
                          for mi in range(4)]
                    srow = ps_sm.tile([1, 512], F32, tag="srow")
                    ets = {}

                    def st_exp(j, tb=tb):
                        st = psum.tile([128, 512], F32, tag="mm",
                                       name="st")
                        for ci in range(4):
                            nc.tensor.matmul(
                                st[:],
                                kT[ci][:, 128 * j:128 * (j + 1)],
                                qT[ci][:, 512 * tb:512 * (tb + 1)],
                                start=(ci == 0), stop=(ci == 3))
                        et = epool.tile([128, 512], MMDT, tag="et",
                                        name="et")
                        nc.scalar.activation(et[:], st[:], AF.Exp,
                                             scale=1.0 / math.sqrt(C))
                        etb = epool.tile([128, 512], mybir.dt.bfloat16,
                                         tag="etb", name="etb")
                        nc.vector.tensor_copy(etb[:], et[:])
                        ets[j] = (et, etb)

                    st_exp(0)
                    if NT > 1:
                        st_exp(1)
                    for j in range(NT):
                        if j + 2 < NT:
                            st_exp(j + 2)
                        et, etb = ets.pop(j)
                        if parts != 'nosums':
                            nc.tensor.matmul(
                                srow[:], ones[:], etb[:],
                                start=(j == 0), stop=(j == NT - 1))
                        for mi in range(4):
                            nc.tensor.matmul(
                                up[mi][:],
                                et[:, 128 * mi:128 * (mi + 1)],
                                xn_rm[:, 512 * j:512 * (j + 1)],
                                start=(j == 0), stop=(j == NT - 1))
                    srow_sb = spool.tile([1, 512], F32, tag="srow_sb")
                    if parts == 'nosums':
                        nc.vector.memset(srow_sb[:], 1.0)
                    else:
                        nc.scalar.activation(srow_sb[:], srow[:],
                                             AF.Identity)
                    dsr = dpool.tile([1, 512], F32, tag="dsr")
                    nc.sync.dma_start(dsr[:], srow_sb[:])
                    scol = spool.tile([128, 4], F32, tag="scol")
                    nc.sync.dma_start(
                        scol[:],
                        dsr[0:1, :].rearrange("a (m p) -> (a p) m", p=128))
                    recip = spool.tile([128, 4], F32, tag="recip")
                    nc.vector.reciprocal(recip[:], scol[:])
                    for mi in range(4):
                        qrm = psum.tile([128, 512], F32, tag="mm")
                        for ci in range(4):
                            nc.tensor.matmul(
                                qrm[:],
                                kT[ci][:, 512 * tb + 128 * mi:
                                       512 * tb + 128 * (mi + 1)],
                                A2[ci][:],
                                start=(ci == 0), stop=(ci == 3))
                        o1 = opool.tile([128, 512], F32, tag="o1")
                        if o1_eng == 'act':
                            # t = up * recip (per-partition) on ACT, then
                            # o1 = t * gv_row on DVE
                            t_ = opool.tile([128, 512], F32, tag="t_")
                            nc.scalar.activation(
                                t_[:], up[mi][:], AF.Identity,
                                scale=recip[:, mi:mi + 1])
                            nc.vector.tensor_tensor(o1[:], t_[:], gv[:],
                                                    ALU.mult)
                        else:
                            nc.vector.scalar_tensor_tensor(
                                o1[:], up[mi][:],
                                recip[:, mi:mi + 1], gv[:],
                                ALU.mult, ALU.mult)
                        y1 = opool.tile([128, 512], F32, tag="y1")
                        nc.vector.tensor_tensor(y1[:], qrm[:], bvq[:], ALU.add)
                        y2 = opool.tile([128, 512], F32, tag="y2")
                        nc.vector.tensor_tensor(y2[:], y1[:], o1[:], ALU.add)
                        row0 = 512 * tb + 128 * mi
                        nc.sync.dma_start(ys[b, row0:row0 + 128, :], y2[:])

        if with_reps:
            rp = wpool.tile([1, 1], I32, tag="reps")
            nc.sync.dma_start(rp[:], repd[:, :])
            n_reps = nc.values_load(rp[0:1, 0:1].to_broadcast((1, 1)))
            with tc.For_i(0, n_reps, 1):
                body()
        else:
            body()

    nc.compile()
    return nc


def prepare_weights(ln1_g, ln1_b, ln2_g, ln2_b, qp, kp, vp, Wq, bq,
                    Wv1, bv1, Wv2, bv2):
    """Host-side folding of all parameter-only math (all fp64 -> fp32)."""
    f8 = np.float64
    ln1_g, ln1_b = f8(ln1_g), f8(ln1_b)
    ln2_g, ln2_b = f8(ln2_g), f8(ln2_b)
    qp, kp, vp = f8(qp), f8(kp), f8(vp)
    Wq, bq = f8(Wq), f8(bq)
    sq = _sigmoid(qp)[0]
    sk = _sigmoid(kp)[0]
    vg = (_sigmoid(vp @ f8(Wv1).T + f8(bv1)) *
          np.tanh(vp @ f8(Wv2).T + f8(bv2)))[0]
    gk = ln2_g * sk
    bk = ln2_b * sk
    gv = ln2_g * vg
    bv = ln2_b * vg
    # q = (xn*g1 + b1) @ Wq.T * sq + bq*sq  => A[c,d] = g1[c]*Wq[d,c]*sq[d]
    A = ln1_g[:, None] * Wq.T * sq[None, :]
    bq2 = (Wq @ ln1_b + bq) * sq
    # substitute xn = (kT - bk)/gk
    A2 = A / gk[:, None]
    bq3 = bq2 - A2.T @ bk

    def pack4(v):
        return np.ascontiguousarray(v.reshape(4, 128).T).astype(np.float32)

    return {
        "A2": A2.astype(np.float32),
        "bvq": np.broadcast_to((bv + bq3).astype(np.float32),
                               (128, C)).copy(),
        "bq3": pack4(bq3),
        "gk": pack4(gk),
        "bk": pack4(bk),
        "gv": np.broadcast_to(gv.astype(np.float32), (128, C)).copy(),
        "bv": np.broadcast_to(bv.astype(np.float32), (128, C)).copy(),
        "ident": np.eye(128, dtype=np.float32),
        "identr": np.eye(128, dtype=np.float32),
        "ones": np.ones((128, 1), dtype=ml_dtypes.bfloat16),
    }


def get_runner(b_loc=B_LOC, t=T, use_f32r=True, with_reps=True, **bkw):
    """Build + jit once; returns run(in_maps) -> list of per-core out dicts.

    Mirrors bass2jax.run_bass_via_pjrt's shard_map path but keeps the jitted
    callable so repeated executions (timing) don't re-trace/re-compile.
    """
    key = (b_loc, t, use_f32r, with_reps, tuple(sorted(bkw.items())))
    if key in _RUNNER_CACHE:
        return _RUNNER_CACHE[key]

    import jax
    from jax.sharding import Mesh, PartitionSpec
    from jax.experimental.shard_map import shard_map
    from concourse import bass2jax, mybir
    from concourse.bass2jax import _bass_exec_p, partition_id_tensor

    nc = build_program(b_loc=b_loc, t=t, use_f32r=use_f32r,
                       with_reps=with_reps, **bkw)
    bass2jax.install_neuronx_cc_hook()

    partition_name = (nc.partition_id_tensor.name
                      if nc.partition_id_tensor else None)
    in_names, out_names, out_avals, zero_shapes = [], [], [], []
    for alloc in nc.m.functions[0].allocations:
        if not isinstance(alloc, mybir.MemoryLocationSet):
            continue
        name = alloc.memorylocations[0].name
        if alloc.kind == "ExternalInput":
            if name != partition_name:
                in_names.append(name)
        elif alloc.kind == "ExternalOutput":
            shape = tuple(alloc.tensor_shape)
            dtype = mybir.dt.np(alloc.dtype)
            out_names.append(name)
            out_avals.append(jax.core.ShapedArray(shape, dtype))
            zero_shapes.append((shape, dtype))
    n_params = len(in_names)
    n_outs = len(out_names)
    all_in = list(in_names) + list(out_names)
    if partition_name is not None:
        all_in.append(partition_name)

    def _body(*args):
        operands = list(args)
        if partition_name is not None:
            operands.append(partition_id_tensor())
        outs = _bass_exec_p.bind(
            *operands,
            out_avals=tuple(out_avals),
            in_names=tuple(all_in),
            out_names=tuple(out_names),
            lowering_input_output_aliases=(),
            sim_require_finite=True,
            sim_require_nnan=True,
            nc=nc,
        )
        return tuple(outs)

    devices = jax.devices()[:N_CORES]
    mesh = Mesh(np.asarray(devices), ("core",))
    in_specs = (PartitionSpec("core"),) * (n_params + n_outs)
    out_specs = (PartitionSpec("core"),) * n_outs
    donate = tuple(range(n_params, n_params + n_outs))
    sharded = jax.jit(
        shard_map(_body, mesh=mesh, in_specs=in_specs, out_specs=out_specs,
                  check_rep=False),
        donate_argnums=donate, keep_unused=True)

    def run(in_maps):
        per_core = [[np.asarray(m[name]) for name in in_names]
                    for m in in_maps]
        concat_in = [
            np.concatenate([per_core[c][i] for c in range(N_CORES)], axis=0)
            for i in range(n_params)
        ]
        concat_zeros = [
            np.zeros((N_CORES * s[0], *s[1:]), d) for (s, d) in zero_shapes
        ]
        out_arrs = sharded(*concat_in, *concat_zeros)
        out_arrs = [np.asarray(a) for a in out_arrs]
        return [
            {name: out_arrs[i].reshape(N_CORES, *out_avals[i].shape)[c]
             for i, name in enumerate(out_names)}
            for c in range(N_CORES)
        ]

    _RUNNER_CACHE[key] = (run, nc)
    return run, nc


def make_in_maps(x, weights, reps=1, with_reps=True):
    maps = []
    for c in range(N_CORES):
        m = {"xs": np.ascontiguousarray(
            x[c * B_LOC:(c + 1) * B_LOC]).astype(np.float32)}
        m.update(weights)
        if with_reps:
            m["reps"] = np.array([[reps]], dtype=np.int32)
        maps.append(m)
    return maps


def kernel(x, ln1_g, ln1_b, ln2_g, ln2_b, qp, kp, vp, Wq, bq,
           Wv1, bv1, Wv2, bv2):
    x = np.asarray(x, dtype=np.float32)
    weights = prepare_weights(ln1_g, ln1_b, ln2_g, ln2_b, qp, kp, vp,
                              Wq, bq, Wv1, bv1, Wv2, bv2)
    run, _ = get_runner()
    in_maps = make_in_maps(x, weights)
    results = run(in_maps)
    out = np.concatenate([results[c]["ys"] for c in range(N_CORES)], axis=0)
    return out.astype(np.float32)
